# revision 9
# baseline (speedup 1.0000x reference)
"""Trainium2 Bass kernel for nn_AdvancedChimeraLayer (B=4, S=8192, D=2048, BUS=128, L=24).

Strategy: sequence-parallel over S across 8 NeuronCores.
  pass 1 (per batch b): SWDGE cast-DMA x tiles to bf16 SBUF (cached), PE-transpose,
          q = x@Wg^T cached in SBUF, pool gate gl on DVE (tensor_tensor_reduce with a
          host-replicated gate row), pool numerator num_b = sum_s sigmoid(gl) * x
          on PE (denominator cancels inside l2norm).
  collective: per-batch AllReduce of num (8KB each), overlapped with later batches.
  summary: l2norm(num_b @ Wpub^T) on device, placed into aug/augT.
  pass 2 (batch b-1, interleaved): scores=q@aug^T -> 25-wide softmax on the free axis
          -> gathered@Wm^T*sig(gate); residual add fused into the PSUM->staging move
          on DVE; SWDGE cast-DMA back out as f32.
"""

import sys

sys.path.insert(0, "/opt/trn_rl_repo")

import math

import ml_dtypes
import numpy as np

import concourse.bass as bass
import concourse.mybir as mybir
import concourse.tile as tile
from concourse import bacc
from concourse import bass_utils

B, S, D, BUS, L = 4, 8192, 2048, 128, 24
NCORES = 8
S_LOC = S // NCORES          # 1024 seq positions per core
TOK = B * S_LOC              # 4096 token rows per core
P = 128
NT = TOK // P                # 32 token tiles per core
TPB = S_LOC // P             # 8 tiles per batch
NCH = D // P                 # 16 d-chunks
LP1 = L + 1                  # 25
BF = mybir.dt.bfloat16
F32 = mybir.dt.float32
BF_NP = ml_dtypes.bfloat16

_CACHE = {}


def _build():
    nc = bacc.Bacc(
        "TRN2", target_bir_lowering=False, debug=False, num_devices=NCORES
    )

    x_d = nc.dram_tensor("x", [TOK, D], F32, kind="ExternalInput")
    wgt_d = nc.dram_tensor("wgt", [P, D], BF, kind="ExternalInput")
    wpgt_d = nc.dram_tensor("wpgt", [P, NCH], BF, kind="ExternalInput")
    wmt_d = nc.dram_tensor("wmt", [P, D], BF, kind="ExternalInput")
    wpubt_d = nc.dram_tensor("wpubt", [P, D], BF, kind="ExternalInput")
    aug0_d = nc.dram_tensor("aug0", [B, LP1, P], BF, kind="ExternalInput")
    augt0_d = nc.dram_tensor("augt0", [B, P, LP1], BF, kind="ExternalInput")
    identb_d = nc.dram_tensor("identb", [P, P], BF, kind="ExternalInput")
    identf_d = nc.dram_tensor("identf", [P, P], F32, kind="ExternalInput")
    onesf_d = nc.dram_tensor("onesf", [P, P], F32, kind="ExternalInput")

    out_d = nc.dram_tensor("out", [TOK, D], F32, kind="ExternalOutput")
    summ_d = nc.dram_tensor("summ", [P, B], F32, kind="ExternalOutput")

    AFT = mybir.ActivationFunctionType
    ALU = mybir.AluOpType

    with tile.TileContext(nc) as tc:
        with (
            tc.tile_pool(name="const", bufs=1) as const,
            tc.tile_pool(name="xbf", bufs=NT) as xbf_pool,
            tc.tile_pool(name="qt", bufs=NT) as qt_pool,
            tc.tile_pool(name="xtsb", bufs=2) as xtsb_pool,
            tc.tile_pool(name="stg", bufs=3) as stg_pool,
            tc.tile_pool(name="sm", bufs=3) as sm_pool,
            tc.tile_pool(name="ps_a", bufs=2, space="PSUM") as ps_a,
            tc.tile_pool(name="ps_b", bufs=2, space="PSUM") as ps_b,
            tc.tile_pool(name="ps_c", bufs=1, space="PSUM") as ps_c,
            tc.tile_pool(name="dram", bufs=4, space="DRAM") as dram_pool,
        ):
            # ---- constants / weights ----
            wgt_sb = const.tile([P, D], BF, tag="wgt")
            wpgt_sb = const.tile([P, NCH], BF, tag="wpgt")
            wmt_sb = const.tile([P, D], BF, tag="wmt")
            wpubt_sb = const.tile([P, D], BF, tag="wpubt")
            identb = const.tile([P, P], BF, tag="identb")
            identf = const.tile([P, P], F32, tag="identf")
            onesf = const.tile([P, P], F32, tag="onesf")
            nc.sync.dma_start(wgt_sb[:], wgt_d[:])
            nc.sync.dma_start(wpgt_sb[:], wpgt_d[:])
            nc.sync.dma_start(wmt_sb[:], wmt_d[:])
            nc.sync.dma_start(wpubt_sb[:], wpubt_d[:])
            nc.sync.dma_start(identb[:], identb_d[:])
            nc.sync.dma_start(identf[:], identf_d[:])
            nc.sync.dma_start(onesf[:], onesf_d[:])

            aug_sb = []
            augt_sb = []
            for b in range(B):
                a = const.tile([LP1, P], BF, tag=f"aug{b}", name=f"aug{b}")
                at = const.tile([P, LP1], BF, tag=f"augt{b}", name=f"augt{b}")
                nc.scalar.dma_start(a[:], aug0_d[b])
                nc.scalar.dma_start(at[:], augt0_d[b])
                aug_sb.append(a)
                augt_sb.append(at)

            num_sb = const.tile([P, B * NCH], F32, tag="num")
            nc.vector.memset(num_sb[:], 0.0)
            numfull = [
                const.tile([P, 2 * NCH], F32, tag=f"numfull{p}", name=f"numfull{p}")
                for p in range(2)
            ]
            numfull_bf = [
                const.tile([P, 2 * NCH], BF, tag=f"numfullbf{p}", name=f"numfullbf{p}")
                for p in range(2)
            ]
            summ_sb = const.tile([P, B], F32, tag="summsb")

            ccin = [
                dram_pool.tile([P, 2 * NCH], F32, tag="ccin", name=f"ccin{p}")
                for p in range(2)
            ]
            ccout = [
                dram_pool.tile([P, 2 * NCH], F32, tag="ccout", name=f"ccout{p}")
                for p in range(2)
            ]

            xbf = []
            qt = []

            def pass1_batch(b):
                for j in range(TPB):
                    i = b * TPB + j
                    r0 = i * P

                    # SWDGE cast-DMA: HBM f32 -> SBUF bf16 (also the x cache)
                    xbf_t = xbf_pool.tile([P, D], BF, tag="xbf")
                    nc.gpsimd.dma_start(xbf_t[:], x_d[r0 : r0 + P, :])
                    xbf.append(xbf_t)

                    # transpose x tile (PE) in two PSUM halves; DVE copies out
                    xtsb_t = xtsb_pool.tile([P, D], BF, tag="xtsb")
                    for h in range(2):
                        xt_ps = ps_a.tile([P, D // 2], BF, tag="xtmod")
                        for k in range(NCH // 2):
                            c = h * (NCH // 2) + k
                            nc.tensor.matmul(
                                xt_ps[:, k * P : (k + 1) * P],
                                xbf_t[:, c * P : (c + 1) * P],
                                identb[:],
                                is_transpose=True,
                            )
                        nc.vector.tensor_copy(
                            xtsb_t[:, h * (D // 2) : (h + 1) * (D // 2)], xt_ps[:]
                        )

                    # qT [128o,128t] / gl [128t,1] / num [128d,1]x16 in one bank
                    mm = ps_b.tile([P, P + 1 + NCH], F32, tag="mmat")
                    for c in range(NCH):
                        nc.tensor.matmul(
                            mm[:, P : P + 1],
                            xtsb_t[:, c * P : (c + 1) * P],
                            wpgt_sb[:, c : c + 1],
                            start=(c == 0),
                            stop=(c == NCH - 1),
                        )
                    g_col = sm_pool.tile([P, 1], BF, tag="g")
                    nc.scalar.activation(g_col[:], mm[:, P : P + 1], AFT.Sigmoid)

                    for c in range(NCH):
                        nc.tensor.matmul(
                            mm[:, 0:P],
                            wgt_sb[:, c * P : (c + 1) * P],
                            xtsb_t[:, c * P : (c + 1) * P],
                            start=(c == 0),
                            stop=(c == NCH - 1),
                        )
                    qt_t = qt_pool.tile([P, P], BF, tag="qt")
                    nc.vector.tensor_copy(qt_t[:], mm[:, 0:P])
                    qt.append(qt_t)

                    for c in range(NCH):
                        nc.tensor.matmul(
                            mm[:, P + 1 + c : P + 2 + c],
                            xbf_t[:, c * P : (c + 1) * P],
                            g_col[:],
                        )
                    nc.vector.tensor_tensor(
                        num_sb[:, b * NCH : (b + 1) * NCH],
                        num_sb[:, b * NCH : (b + 1) * NCH],
                        mm[:, P + 1 : P + 1 + NCH],
                        ALU.add,
                    )

            def collective_pair(p):
                nc.gpsimd.dma_start(
                    ccin[p][:], num_sb[:, p * 2 * NCH : (p + 1) * 2 * NCH]
                )
                nc.gpsimd.collective_compute(
                    "AllReduce",
                    ALU.add,
                    ins=[ccin[p].opt()],
                    outs=[ccout[p].opt()],
                    replica_groups=[list(range(NCORES))],
                )
                nc.sync.dma_start(numfull[p][:], ccout[p][:])

            def summary_batch(b):
                # summary_b = l2norm(num_b @ Wpub^T)
                p, hb = b // 2, b % 2
                if hb == 0:
                    nc.vector.tensor_copy(numfull_bf[p][:], numfull[p][:])
                raw_ps = ps_b.tile([P, 1], F32, tag="mmat")
                for c in range(NCH):
                    nc.tensor.matmul(
                        raw_ps[:],
                        wpubt_sb[:, c * P : (c + 1) * P],
                        numfull_bf[p][:, hb * NCH + c : hb * NCH + c + 1],
                        start=(c == 0),
                        stop=(c == NCH - 1),
                    )
                sq_sb = sm_pool.tile([P, 1], F32, tag="sq")
                nc.scalar.activation(sq_sb[:], raw_ps[:], AFT.Square)
                n2_ps = ps_b.tile([1, 1], F32, tag="mmat")
                nc.tensor.matmul(n2_ps[:], sq_sb[:], onesf[:, 0:1])
                nrm_sb = sm_pool.tile([1, 1], F32, tag="nrm")
                nc.scalar.activation(nrm_sb[:], n2_ps[:], AFT.Sqrt)
                rs_sb = sm_pool.tile([1, 1], F32, tag="rs")
                nc.vector.reciprocal(rs_sb[:], nrm_sb[:])
                rsb_ps = ps_b.tile([P, 1], F32, tag="mmat")
                nc.tensor.matmul(rsb_ps[:], onesf[0:1, :], rs_sb[:])
                rsb_sb = sm_pool.tile([P, 1], F32, tag="rsb")
                nc.scalar.copy(rsb_sb[:], rsb_ps[:])
                nc.vector.tensor_tensor(
                    summ_sb[:, b : b + 1], raw_ps[:], rsb_sb[:], ALU.mult
                )
                nc.vector.tensor_copy(augt_sb[b][:, 0:1], summ_sb[:, b : b + 1])
                srow_ps = ps_b.tile([1, P], F32, tag="mmat")
                nc.tensor.matmul(
                    srow_ps[:], summ_sb[:, b : b + 1], identf[:], is_transpose=True
                )
                nc.vector.tensor_copy(aug_sb[b][0:1, :], srow_ps[:])

            def pass2_batch(b):
                # batched attention: all 8 tiles of the batch share wide PSUM
                # tiles so each engine does 8 tiles of work per visit
                i0 = b * TPB
                scores8 = ps_c.tile([P, TPB * LP1], F32, tag="sc8")
                for j in range(TPB):
                    nc.tensor.matmul(
                        scores8[:, j * LP1 : (j + 1) * LP1],
                        qt[i0 + j][:],
                        augt_sb[b][:],
                    )
                attn_e8 = sm_pool.tile([P, TPB * LP1], BF, tag="attne8")
                sumexp8 = sm_pool.tile([P, TPB], F32, tag="sumexp8")
                for j in range(TPB):
                    nc.scalar.activation(
                        attn_e8[:, j * LP1 : (j + 1) * LP1],
                        scores8[:, j * LP1 : (j + 1) * LP1],
                        AFT.Exp,
                        accum_out=sumexp8[:, j : j + 1],
                    )
                recip8 = sm_pool.tile([P, TPB], F32, tag="recip8")
                nc.vector.reciprocal(recip8[:], sumexp8[:])
                attn_n8 = sm_pool.tile([P, TPB * LP1], BF, tag="attnn8")
                for j in range(TPB):
                    nc.vector.tensor_scalar(
                        attn_n8[:, j * LP1 : (j + 1) * LP1],
                        attn_e8[:, j * LP1 : (j + 1) * LP1],
                        recip8[:, j : j + 1],
                        None,
                        ALU.mult,
                    )
                attnt8 = ps_c.tile([LP1, TPB * P], BF, tag="att8")
                for j in range(TPB):
                    nc.tensor.matmul(
                        attnt8[:, j * P : (j + 1) * P],
                        attn_n8[:, j * LP1 : (j + 1) * LP1],
                        identb[:],
                        is_transpose=True,
                    )
                attnt8_sb = sm_pool.tile([LP1, TPB * P], BF, tag="attnt8")
                nc.vector.tensor_copy(attnt8_sb[:], attnt8[:])
                gath8 = ps_c.tile([P, TPB * P], F32, tag="ga8")
                for j in range(TPB):
                    nc.tensor.matmul(
                        gath8[:, j * P : (j + 1) * P],
                        aug_sb[b][:],
                        attnt8_sb[:, j * P : (j + 1) * P],
                    )
                gath8_sb = sm_pool.tile([P, TPB * P], BF, tag="gath8")
                nc.vector.tensor_copy(gath8_sb[:], gath8[:])

                for j in range(TPB):
                    i = i0 + j
                    r0 = i * P
                    stg = stg_pool.tile([P, D], BF, tag="stg")
                    for q in range(4):
                        f0 = q * 512
                        mod_ps = ps_a.tile([P, 512], F32, tag="xtmod")
                        nc.tensor.matmul(
                            mod_ps[:],
                            gath8_sb[:, j * P : (j + 1) * P],
                            wmt_sb[:, f0 : f0 + 512],
                            start=True,
                            stop=False,
                        )
                        nc.tensor.matmul(
                            mod_ps[:],
                            identb[:],
                            xbf[i][:, f0 : f0 + 512],
                            start=False,
                            stop=True,
                        )
                        if q % 2 == 0:
                            nc.scalar.copy(stg[:, f0 : f0 + 512], mod_ps[:])
                        else:
                            nc.vector.tensor_copy(stg[:, f0 : f0 + 512], mod_ps[:])
                    nc.gpsimd.dma_start(out_d[r0 : r0 + P, :], stg[:])

            # pass-1 feeds paired collectives ASAP; pass-2 b0/b1 interleave
            # ahead of pass-1 b3 so out-DMAs start early
            pass1_batch(0)
            pass1_batch(1)
            collective_pair(0)
            pass1_batch(2)
            summary_batch(0)
            pass2_batch(0)
            summary_batch(1)
            pass2_batch(1)
            pass1_batch(3)
            collective_pair(1)
            summary_batch(2)
            pass2_batch(2)
            summary_batch(3)
            pass2_batch(3)

            nc.sync.dma_start(summ_d[:], summ_sb[:])

    nc.compile()
    return nc


def _get_nc():
    if "nc" not in _CACHE:
        _CACHE["nc"] = _build()
    return _CACHE["nc"]


def _prep_inputs(x, bus_cache, W_publish, W_gather_q, W_modulate, W_pool_gate, gate):
    x = np.asarray(x, dtype=np.float32)
    bus_cache = np.asarray(bus_cache, dtype=np.float32)
    sg = 1.0 / (1.0 + math.exp(-float(np.asarray(gate).reshape(-1)[0])))
    scale = 1.0 / math.sqrt(BUS)

    # lhsT chunk layouts: w[p, c*128+o] = W[o, c*128+p]
    def chunked_T(w):  # w: [BUS, D] -> [128, D]
        return (
            np.ascontiguousarray(w.T.reshape(NCH, P, BUS).transpose(1, 0, 2))
            .reshape(P, D)
        )

    wgt = chunked_T(np.asarray(W_gather_q, np.float32) * scale).astype(BF_NP)
    wpubt = chunked_T(np.asarray(W_publish, np.float32)).astype(BF_NP)
    wpgt = (
        np.asarray(W_pool_gate, np.float32).reshape(NCH, P).T.astype(BF_NP)
    )  # [128, 16]
    wmt = (np.asarray(W_modulate, np.float32).T * sg).astype(BF_NP)  # [BUS, D]

    aug0 = np.zeros((B, LP1, P), np.float32)
    aug0[:, 1:, :] = bus_cache
    augt0 = np.zeros((B, P, LP1), np.float32)
    augt0[:, :, 1:] = bus_cache.transpose(0, 2, 1)

    shared = {
        "wgt": wgt,
        "wpgt": wpgt,
        "wmt": wmt,
        "wpubt": wpubt,
        "aug0": aug0.astype(BF_NP),
        "augt0": augt0.astype(BF_NP),
        "identb": np.eye(P, dtype=np.float32).astype(BF_NP),
        "identf": np.eye(P, dtype=np.float32),
        "onesf": np.ones((P, P), np.float32),
    }
    in_maps = []
    for c in range(NCORES):
        shard = np.ascontiguousarray(
            x[:, c * S_LOC : (c + 1) * S_LOC, :]
        ).reshape(TOK, D)
        in_maps.append({"x": shard, **shared})
    return in_maps


def _run(inputs, trace=False):
    nc = _get_nc()
    in_maps = _prep_inputs(**inputs)
    res = bass_utils.run_bass_kernel_spmd(
        nc, in_maps, core_ids=list(range(NCORES)), trace=trace
    )
    x = np.asarray(inputs["x"], np.float32)
    bus_cache = np.asarray(inputs["bus_cache"], np.float32)
    x_out = np.empty((B, S, D), np.float32)
    for c in range(NCORES):
        x_out[:, c * S_LOC : (c + 1) * S_LOC, :] = res.results[c]["out"].reshape(
            B, S_LOC, D
        )
    summary = np.asarray(res.results[0]["summ"], np.float32).T  # [B, BUS]
    new_cache = np.concatenate([bus_cache, summary[:, None, :]], axis=1)
    return (x_out, new_cache), res


def kernel(**inputs):
    (x_out, new_cache), _ = _run(inputs, trace=False)
    return x_out, new_cache
